# revision 20
# baseline (speedup 1.0000x reference)
"""Trainium2 Bass kernel for BertWithAdaThresholdLocContextPooling.

Strategy: pure data parallel over batch (B=16 -> 2 batches per core x 8 cores).
Each core:
  - gathers mention rows of sequence_output / attention via indirect DMA
    (only ~0.2MB of the 12.6MB attention shard is ever read from HBM),
  - logsumexp-pools mention embeddings, mean-pools attention rows,
  - computes the localized-context attention rs = seq^T @ ht,
  - runs the two extractor GEMVs (bf16 data, fp32 accumulate),
  - forms the grouped bilinear via PE replication matmuls,
  - applies the classifier Wb.
Weights are replicated to all cores; the host pre-transposes/casts them and
packs small constants so each core issues only a handful of large DMAs.
"""

import sys

for _p in ("/opt/trn_rl_repo",):
    if _p not in sys.path:
        sys.path.insert(0, _p)

import numpy as np
import ml_dtypes

import concourse.bacc as bacc
import concourse.bass as bass
import concourse.mybir as mybir
from concourse.tile import TileContext
from concourse.bass_utils import run_bass_kernel_spmd

F32 = mybir.dt.float32
BF16 = mybir.dt.bfloat16
I32 = mybir.dt.int32
AF = mybir.ActivationFunctionType
ALU = mybir.AluOpType

B, L, HID = 16, 512, 768
HEADS, M = 12, 4
EMB, BLK, NER, NCLS = 768, 8, 6, 97
NCORES = 8
BPC = B // NCORES          # batches per core = 2
CAT = 2 * HID + NER        # 1542
KCH = 12                   # full 128-row contraction chunks of CAT
NEMB = EMB // 128          # 6 chunks of EMB
NL = L // 128              # 4 chunks of L
NBL = EMB * BLK // 128     # 48 classifier contraction chunks

# packed-constant layouts
# CIDX [96, 99] f32 (critical path): rep8 [0:8,0:96] | baseA [0:96,96:98]
#                                    | baseS [0:16,98:99]
CIDX_COLS = 99
# CID2 [128, 129] f32: identity [0:128,0:128] | bbc [0:97,128:129]
CID2_COLS = 129
# CBA [96, 29] bf16 (early): selE [0:16,0:4] | selA [0:96,4:28] | w12 [0:12,28:29]
CBA_COLS = 29
# CBB [128, 2568] bf16 (late): rys [0:128,0:1024] | bhr [0:1,1024:1792]
#   | btr [0:1,1792:2560] | selbh [0:1,2560:2564] | selbt [0:1,2564:2568]
RYS0 = 0
BHR0 = 1024
BTR0 = BHR0 + 768
SELBH0 = BTR0 + 768
CBB_COLS = SELBH0 + 8

_cache = {}


def _build_constants():
    selE = np.zeros((4 * M, 4), np.float32)
    for k in range(4 * M):
        selE[k, k // M] = 1.0
    selA = np.zeros((2 * M * HEADS, 2 * HEADS), np.float32)
    for i in range(2):
        for m in range(M):
            for h in range(HEADS):
                selA[i * M * HEADS + m * HEADS + h, i * HEADS + h] = 1.0 / M
    rep8 = np.zeros((2 * M, 2 * M * HEADS), np.float32)
    for q in range(2 * M * HEADS):
        rep8[q // HEADS, q] = 1.0
    baseA = np.zeros((2 * M * HEADS, BPC), np.float32)
    for q in range(2 * M * HEADS):
        for b in range(BPC):
            baseA[q, b] = (b * HEADS + q % HEADS) * L + 1
    baseS = np.zeros((4 * M, 1), np.float32)
    for k in range(4 * M):
        baseS[k, 0] = (k // (2 * M)) * L + 1

    cidx = np.zeros((96, CIDX_COLS), np.float32)
    cidx[0:8, 0:96] = rep8
    cidx[0:96, 96:98] = baseA
    cidx[0:16, 98:99] = baseS
    cid2 = np.zeros((128, CID2_COLS), np.float32)
    cid2[0:128, 0:128] = np.eye(128)
    # bbc filled per-call (bias input)

    cba = np.zeros((96, CBA_COLS), ml_dtypes.bfloat16)
    cba[0:16, 0:4] = selE
    cba[0:96, 4:28] = selA
    cba[0:12, 28:29] = 1.0 / HEADS
    cbb = np.zeros((128, CBB_COLS), ml_dtypes.bfloat16)
    for y in range(BLK):
        for p in range(128):
            cbb[(p // BLK) * BLK + y, RYS0 + y * 128 + p] = 1.0
    cbb[0:1, SELBH0:SELBH0 + 4] = np.array([1.0, 0.0, 1.0, 0.0])
    cbb[0:1, SELBH0 + 4:SELBH0 + 8] = np.array([0.0, 1.0, 0.0, 1.0])

    perm = np.empty(EMB * BLK, np.int64)
    for cch in range(NEMB):
        for y in range(BLK):
            for p in range(128):
                g = cch * 16 + p // BLK
                x = p % BLK
                perm[(cch * BLK + y) * 128 + p] = g * 64 + x * BLK + y
    return {"cidx": cidx, "cid2": cid2, "cba": cba, "cbb": cbb, "perm": perm}


def _build_program(stage=99):
    nc = bacc.Bacc("TRN2", target_bir_lowering=False, debug=False)

    seq_h = nc.dram_tensor("seq", [BPC * L, HID], BF16, kind="ExternalInput")
    attn_h = nc.dram_tensor("attn", [BPC * HEADS * L, L], BF16, kind="ExternalInput")
    pos_h = nc.dram_tensor("pos", [4 * M, 1], I32, kind="ExternalInput")
    posb_hs = [
        nc.dram_tensor(f"posb{b}", [2 * M, 1], I32, kind="ExternalInput")
        for b in range(BPC)
    ]
    ner_h = nc.dram_tensor("ner", [NER, 4], F32, kind="ExternalInput")
    whs_h = nc.dram_tensor("whs", [128, KCH * EMB + EMB], BF16, kind="ExternalInput")
    wts_h = nc.dram_tensor("wts", [128, KCH * EMB + EMB], BF16, kind="ExternalInput")
    wbs_h = nc.dram_tensor("wbs", [128, NBL * 128], BF16, kind="ExternalInput")
    cidx_h = nc.dram_tensor("cidx", [96, CIDX_COLS], F32, kind="ExternalInput")
    cid2_h = nc.dram_tensor("cid2", [128, CID2_COLS], F32, kind="ExternalInput")
    cba_h = nc.dram_tensor("cba", [96, CBA_COLS], BF16, kind="ExternalInput")
    cbb_h = nc.dram_tensor("cbb", [128, CBB_COLS], BF16, kind="ExternalInput")
    out_h = nc.dram_tensor("logitsT", [NCLS, BPC], F32, kind="ExternalOutput")

    with TileContext(nc) as tc:
        with (
            tc.tile_pool(name="const", bufs=1) as cp,
            tc.tile_pool(name="data", bufs=1) as dp,
            tc.tile_pool(name="psbig", bufs=1, space="PSUM") as psb,
            tc.tile_pool(name="psea", bufs=2, space="PSUM") as pse,
            tc.tile_pool(name="pssm", bufs=3, space="PSUM") as pss,
        ):
            # ---- critical small loads first (sync queue) ----
            cidx = cp.tile([96, CIDX_COLS], F32)
            nc.sync.dma_start(cidx[:], cidx_h[:])
            posi = dp.tile([4 * M, 1], I32)
            nc.sync.dma_start(posi[:], pos_h[:])
            posbi = []
            for b in range(BPC):
                t = dp.tile([2 * M, 1], I32, tag=f"posbi{b}")
                nc.sync.dma_start(t[:], posb_hs[b][:])
                posbi.append(t)
            cba = cp.tile([96, CBA_COLS], BF16)
            nc.sync.dma_start(cba[:], cba_h[:])
            cid2 = cp.tile([128, CID2_COLS], F32)
            nc.sync.dma_start(cid2[:], cid2_h[:])
            rep8 = cidx[0:8, 0:96]
            baseA = cidx[0:96, 96:98]
            baseS = cidx[0:16, 98:99]
            bbc = cid2[0:97, 128:129]
            selE = cba[0:16, 0:4]
            selA = cba[0:96, 4:28]
            w12 = cba[0:12, 28:29]
            idf = cid2[:, 0:128]

            # ---- index computation ----
            posf = dp.tile([4 * M, 1], F32)
            nc.vector.tensor_copy(posf[:], posi[:])
            idxsf = dp.tile([4 * M, 1], F32)
            nc.vector.tensor_add(idxsf[:], posf[:], baseS)
            idxs = dp.tile([4 * M, 1], I32)
            nc.vector.tensor_copy(idxs[:], idxsf[:])

            idxa = []
            for b in range(BPC):
                posbf = dp.tile([2 * M, 1], F32, tag=f"posbf{b}")
                nc.vector.tensor_copy(posbf[:], posbi[b][:])
                ps_idx = pss.tile([2 * M * HEADS, 1], F32, tag="sm")
                nc.tensor.matmul(ps_idx[:], lhsT=rep8, rhs=posbf[:],
                                 start=True, stop=True)
                idxaf = dp.tile([2 * M * HEADS, 1], F32, tag=f"idxaf{b}")
                nc.vector.tensor_add(idxaf[:], ps_idx[:], baseA[:, b:b + 1])
                ia = dp.tile([2 * M * HEADS, 1], I32, tag=f"idxa{b}")
                nc.vector.tensor_copy(ia[:], idxaf[:])
                idxa.append(ia)

            # ---- gathers ----
            sg = dp.tile([4 * M, HID], BF16)
            g0 = nc.gpsimd.indirect_dma_start(
                out=sg[:], out_offset=None, in_=seq_h[:],
                in_offset=bass.IndirectOffsetOnAxis(ap=idxs[:, :1], axis=0))
            at = []
            for b in range(BPC):
                t = dp.tile([2 * M * HEADS, L], BF16, tag=f"at{b}")
                g = nc.gpsimd.indirect_dma_start(
                    out=t[:], out_offset=None, in_=attn_h[:],
                    in_offset=bass.IndirectOffsetOnAxis(ap=idxa[b][:, :1], axis=0))
                at.append(t)

            # ---- bulk loads, held behind the gathers so the tiny critical
            # transfers are not starved of SDMA bandwidth ----
            from concourse.tile_rust import add_dep_helper
            big = []
            whsf = cp.tile([128, KCH * EMB + EMB], BF16)
            big.append(nc.scalar.dma_start(whsf[:], whs_h[:]))
            whs = whsf[:, 0:KCH * EMB]
            whn = whsf[0:NER, KCH * EMB:KCH * EMB + EMB]
            wtsf = cp.tile([128, KCH * EMB + EMB], BF16)
            big.append(nc.scalar.dma_start(wtsf[:], wts_h[:]))
            wts = wtsf[:, 0:KCH * EMB]
            wtn = wtsf[0:NER, KCH * EMB:KCH * EMB + EMB]
            seqt = []
            for b in range(BPC):
                t = dp.tile([128, NL * HID], BF16, tag=f"seq{b}")
                big.append(nc.sync.dma_start(
                    t[:].rearrange("p (c d) -> p c d", c=NL),
                    seq_h[b * L:(b + 1) * L, :].rearrange("(c p) d -> p c d", p=128)))
                seqt.append(t)
            wbs = cp.tile([128, NBL * 128], BF16)
            big.append(nc.scalar.dma_start(wbs[:], wbs_h[:]))
            cbb = cp.tile([128, CBB_COLS], BF16)
            big.append(nc.sync.dma_start(cbb[:], cbb_h[:]))
            rys = cbb[:, RYS0:RYS0 + 1024]
            bhr = cbb[0:1, BHR0:BHR0 + EMB]
            btr = cbb[0:1, BTR0:BTR0 + EMB]
            selbh = cbb[0:1, SELBH0:SELBH0 + 4]
            selbt = cbb[0:1, SELBH0 + 4:SELBH0 + 8]
            ner4f = dp.tile([NER, 4], F32)
            big.append(nc.sync.dma_start(ner4f[:], ner_h[:]))
            ner4 = dp.tile([NER, 4], BF16)
            nc.vector.tensor_copy(ner4[:], ner4f[:])
            for d in big:
                add_dep_helper(d.ins, g0.ins,
                               reason="bulk loads yield SDMA to gathers")

            if stage < 1:
                lg = dp.tile([NCLS, BPC], F32)
                nc.vector.memset(lg[:], 0.0)
                nc.vector.tensor_copy(lg[0:4 * M, 0:1], sg[:, 0:1])
                nc.vector.tensor_copy(lg[0:2 * M * HEADS - 31, 1:2], at[0][0:2 * M * HEADS - 31, 0:1])
                nc.sync.dma_start(out_h[:], lg[:])
                return _finish(nc)

            # ---- entity embeddings: log-sum-exp over mentions ----
            exps = dp.tile([4 * M, HID], BF16)
            nc.scalar.activation(exps[:], sg[:], AF.Exp)
            ps_e = psb.tile([4, HID], F32, tag="big")
            for n0, nl_ in ((0, 512), (512, 256)):
                nc.tensor.matmul(ps_e[:, n0:n0 + nl_], lhsT=selE,
                                 rhs=exps[:, n0:n0 + nl_], start=True, stop=True)
            ent = dp.tile([4, HID], F32)
            nc.scalar.activation(ent[:], ps_e[:], AF.Ln)
            ps_et = pss.tile([128, 4 * NEMB], F32, tag="sm")
            for c in range(NEMB):
                nc.tensor.transpose(ps_et[:, c * 4:(c + 1) * 4],
                                    ent[:, c * 128:(c + 1) * 128], idf[0:4, 0:4])
            entT = dp.tile([128, 4 * NEMB], BF16)
            nc.vector.tensor_copy(entT[:], ps_et[:])

            # ---- entity attention pooling + context vector ----
            htc = []
            for b in range(BPC):
                ps_eah = pse.tile([HEADS, L], F32, tag="ea")
                nc.tensor.matmul(ps_eah[:], lhsT=selA[:, 0:HEADS], rhs=at[b][:],
                                 start=True, stop=True)
                ps_eat = pse.tile([HEADS, L], F32, tag="ea")
                nc.tensor.matmul(ps_eat[:], lhsT=selA[:, HEADS:2 * HEADS],
                                 rhs=at[b][:], start=True, stop=True)
                eah = dp.tile([HEADS, L], F32, tag=f"eah{b}")
                nc.vector.tensor_copy(eah[:], ps_eah[:])
                prd = dp.tile([HEADS, L], BF16, tag=f"prd{b}")
                nc.vector.tensor_tensor(out=prd[:], in0=eah[:], in1=ps_eat[:],
                                        op=ALU.mult)
                ps_ht = pss.tile([1, L], F32, tag="sm")
                nc.tensor.matmul(ps_ht[:], lhsT=w12, rhs=prd[:],
                                 start=True, stop=True)
                sm = dp.tile([1, 1], F32, tag=f"sm{b}")
                nc.vector.reduce_sum(sm[:], ps_ht[:], axis=mybir.AxisListType.X)
                den = dp.tile([1, 1], F32, tag=f"den{b}")
                nc.vector.tensor_scalar_add(den[:], sm[:], 1e-5)
                rcp = dp.tile([1, 1], F32, tag=f"rcp{b}")
                nc.vector.reciprocal(rcp[:], den[:])
                htn = dp.tile([1, L], F32, tag=f"htn{b}")
                nc.vector.tensor_scalar_mul(htn[:], ps_ht[:], rcp[:, :1])
                ps_htc = pss.tile([128, NL], F32, tag="sm")
                for c in range(NL):
                    nc.tensor.transpose(ps_htc[:, c:c + 1],
                                        htn[:, c * 128:(c + 1) * 128],
                                        idf[0:1, 0:1])
                h = dp.tile([128, NL], BF16, tag=f"htc{b}")
                nc.vector.tensor_copy(h[:], ps_htc[:])
                htc.append(h)

            if stage < 2:
                lg = dp.tile([NCLS, BPC], F32)
                nc.vector.memset(lg[:], 0.0)
                nc.vector.tensor_copy(lg[0:97, 0:1], entT[0:97, 0:1])
                nc.vector.tensor_copy(lg[0:97, 1:2], htc[0][0:97, 0:1])
                nc.sync.dma_start(out_h[:], lg[:])
                return _finish(nc)

            # ---- rs = seq^T @ ht  (column form) ----
            ps_rsc = pss.tile([128, NEMB * BPC], F32, tag="sm")
            for b in range(BPC):
                for d in range(NEMB):
                    for c in range(NL):
                        nc.tensor.matmul(
                            ps_rsc[:, d * BPC + b:d * BPC + b + 1],
                            lhsT=seqt[b][:, c * HID + d * 128:c * HID + (d + 1) * 128],
                            rhs=htc[b][:, c:c + 1],
                            start=(c == 0), stop=(c == NL - 1))
            rsc = dp.tile([128, 4 * NEMB], BF16)
            nc.vector.tensor_copy(
                rsc[:].rearrange("p (r b m) -> p r b m", r=NEMB, b=BPC),
                ps_rsc[:].rearrange("p (r b) -> p r b", r=NEMB)
                .unsqueeze(3).broadcast_to([128, NEMB, BPC, 2]))

            # ---- extractor GEMVs:  [4,768] = cat4^T @ W^T  ----
            def cat_chunk(j):
                if j < NEMB:
                    return entT[:, j * 4:(j + 1) * 4]
                if j < 2 * NEMB:
                    return rsc[:, (j - NEMB) * 4:(j - NEMB + 1) * 4]
                return ner4[:]

            t4 = []
            for wi, (ws, wn, selb, br) in enumerate(
                    ((whs, whn, selbh, bhr), (wts, wtn, selbt, btr))):
                ps_w = psb.tile([4, EMB], F32, tag="big")
                for n0, nl_ in ((0, 512), (512, 256)):
                    for j in range(KCH + 1):
                        lhsT = cat_chunk(j)
                        rhs = (ws[:, j * EMB + n0:j * EMB + n0 + nl_] if j < KCH
                               else wn[:, n0:n0 + nl_])
                        nc.tensor.matmul(ps_w[:, n0:n0 + nl_], lhsT=lhsT, rhs=rhs,
                                         start=(j == 0), stop=False)
                    nc.tensor.matmul(ps_w[:, n0:n0 + nl_], lhsT=selb,
                                     rhs=br[:, n0:n0 + nl_], start=False, stop=True)
                t = dp.tile([4, EMB], F32, tag=f"t4_{wi}")
                nc.scalar.activation(t[:], ps_w[:], AF.Tanh)
                t4.append(t)

            if stage < 3:
                lg = dp.tile([NCLS, BPC], F32)
                nc.vector.memset(lg[:], 0.0)
                nc.vector.tensor_copy(lg[0:4, 0:2], t4[0][:, 0:2])
                nc.vector.tensor_copy(lg[0:4, 1:2], t4[1][:, 0:1])
                nc.vector.tensor_copy(lg[0:89, 0:1], rsc[0:89, 0:1])
                nc.sync.dma_start(out_h[:], lg[:])
                return _finish(nc)

            # ---- transpose hs2/ts2 to columns ----
            ps_a = pss.tile([128, 4 * NEMB], F32, tag="sm")
            ps_b2 = pss.tile([128, 4 * NEMB], F32, tag="sm")
            for c in range(NEMB):
                nc.tensor.transpose(ps_a[:, c * 4:(c + 1) * 4],
                                    t4[0][:, c * 128:(c + 1) * 128], idf[0:4, 0:4])
                nc.tensor.transpose(ps_b2[:, c * 4:(c + 1) * 4],
                                    t4[1][:, c * 128:(c + 1) * 128], idf[0:4, 0:4])
            h2t = dp.tile([128, 4 * NEMB], BF16)
            nc.vector.tensor_copy(
                h2t[:].rearrange("p (c b) -> p c b", c=NEMB)[:, :, 0:4:2],
                ps_a[:].rearrange("p (c b) -> p c b", c=NEMB)[:, :, 0:4:2])
            nc.vector.tensor_copy(
                h2t[:].rearrange("p (c b) -> p c b", c=NEMB)[:, :, 1:4:2],
                ps_b2[:].rearrange("p (c b) -> p c b", c=NEMB)[:, :, 1:4:2])

            if stage < 4:
                lg = dp.tile([NCLS, BPC], F32)
                nc.vector.memset(lg[:], 0.0)
                nc.vector.tensor_copy(lg[0:97, 0:2], h2t[0:97, 0:2])
                nc.sync.dma_start(out_h[:], lg[:])
                return _finish(nc)

            # ---- grouped bilinear + classifier ----
            # ts-replication: out col layout (y, c, b) = y*12 + c*2 + b
            ps_t2x = pss.tile([128, BLK * NEMB * BPC], F32, tag="sm")
            tscols = h2t[:].rearrange("p (c b) -> p c b", c=NEMB)[:, :, 1:4:2]
            for y in range(BLK):
                nc.tensor.matmul(
                    ps_t2x[:, y * 12:(y + 1) * 12]
                    .rearrange("p (c b) -> p c b", c=NEMB),
                    lhsT=rys[:, y * 128:(y + 1) * 128],
                    rhs=tscols, start=True, stop=True)
            if stage < 5:
                lg = dp.tile([NCLS, BPC], F32)
                nc.vector.memset(lg[:], 0.0)
                nc.vector.tensor_copy(lg[0:97, 0:2], ps_t2x[0:97, 0:2])
                nc.sync.dma_start(out_h[:], lg[:])
                return _finish(nc)

            blt = dp.tile([128, NEMB * 16], BF16)
            for c in range(NEMB):
                nc.vector.tensor_tensor(
                    out=blt[:, c * 16:(c + 1) * 16]
                    .rearrange("p (y b) -> p y b", y=BLK),
                    in0=h2t[:, c * 4:c * 4 + 4:2].unsqueeze(1)
                        .broadcast_to([128, BLK, 2]),
                    in1=ps_t2x[:].rearrange("p (y c b) -> p y c b", y=BLK, c=NEMB)
                    [:, :, c, :],
                    op=ALU.mult)
            if stage < 6:
                lg = dp.tile([NCLS, BPC], F32)
                nc.vector.memset(lg[:], 0.0)
                nc.vector.tensor_copy(lg[0:97, 0:2], blt[0:97, 0:2])
                nc.sync.dma_start(out_h[:], lg[:])
                return _finish(nc)

            ps_l = pss.tile([NCLS, BPC], F32, tag="sm")
            for c in range(NEMB):
                for y in range(BLK):
                    k = c * BLK + y
                    nc.tensor.matmul(ps_l[:], lhsT=wbs[:, k * 128:k * 128 + NCLS],
                                     rhs=blt[:, c * 16 + y * 2:c * 16 + y * 2 + 2],
                                     start=(k == 0), stop=(k == NBL - 1))
            lg = dp.tile([NCLS, BPC], F32)
            if stage < 7:
                nc.vector.memset(lg[:], 0.0)
                nc.vector.tensor_copy(lg[0:1, 0:1], ps_l[0:1, 0:1])
            else:
                nc.vector.tensor_scalar_add(lg[:], ps_l[:], bbc[:, :1])
            nc.sync.dma_start(out_h[:], lg[:])

    return _finish(nc)


def _finish(nc):
    return nc


def _get_program():
    if "nc" not in _cache:
        nc = _build_program()
        nc.finalize()
        _cache["nc"] = nc
        _cache["consts"] = _build_constants()
    return _cache["nc"], _cache["consts"]


def kernel(sequence_output, attention, entity_pos, hs_ner_tags, ts_ner_tags,
           Wh, bh, Wt, bt, Wb, bb):
    nc, c = _get_program()

    seq = np.asarray(sequence_output, dtype=np.float32).astype(ml_dtypes.bfloat16)
    attn = np.asarray(attention, dtype=np.float32).astype(ml_dtypes.bfloat16)
    pos = np.asarray(entity_pos).astype(np.int32)
    nh = np.asarray(hs_ner_tags, dtype=np.float32)
    nt = np.asarray(ts_ner_tags, dtype=np.float32)
    whT = np.ascontiguousarray(np.asarray(Wh, dtype=np.float32).T).astype(ml_dtypes.bfloat16)
    wtT = np.ascontiguousarray(np.asarray(Wt, dtype=np.float32).T).astype(ml_dtypes.bfloat16)
    wbT = np.ascontiguousarray(np.asarray(Wb, dtype=np.float32).T)[c["perm"]]
    wbT = np.pad(wbT, ((0, 0), (0, 128 - NCLS))).astype(ml_dtypes.bfloat16)

    def sbuf_image(w, extra):
        main = w[0:KCH * 128].reshape(KCH, 128, EMB).transpose(1, 0, 2).reshape(128, KCH * EMB)
        img = np.zeros((128, KCH * EMB + EMB), ml_dtypes.bfloat16)
        img[:, 0:KCH * EMB] = main
        img[0:NER, KCH * EMB:] = extra
        return img

    whs = sbuf_image(whT, whT[KCH * 128:CAT])
    wts = sbuf_image(wtT, wtT[KCH * 128:CAT])
    wbs = wbT.reshape(NBL, 128, 128).transpose(1, 0, 2).reshape(128, NBL * 128)
    wbs = np.ascontiguousarray(wbs)

    c["cbb"][0:1, BHR0:BHR0 + EMB] = np.asarray(bh, np.float32).reshape(1, EMB)
    c["cbb"][0:1, BTR0:BTR0 + EMB] = np.asarray(bt, np.float32).reshape(1, EMB)
    cid2 = c["cid2"].copy()
    cid2[0:97, 128] = np.asarray(bb, np.float32)

    in_maps = []
    for core in range(NCORES):
        b0 = core * BPC
        pc = np.ascontiguousarray(pos[b0:b0 + BPC])          # [2,2,M]
        ner = np.stack([nh[b0], nt[b0], nh[b0 + 1], nt[b0 + 1]], axis=1)
        im = {
            "seq": np.ascontiguousarray(seq[b0:b0 + BPC]).reshape(BPC * L, HID),
            "attn": np.ascontiguousarray(attn[b0:b0 + BPC]).reshape(BPC * HEADS * L, L),
            "pos": pc.reshape(4 * M, 1),
            "ner": np.ascontiguousarray(ner.astype(np.float32)),
            "whs": whs, "wts": wts, "wbs": wbs,
            "cidx": c["cidx"], "cid2": cid2, "cba": c["cba"], "cbb": c["cbb"],
        }
        for b in range(BPC):
            im[f"posb{b}"] = np.ascontiguousarray(pc[b].reshape(2 * M, 1))
        in_maps.append(im)

    res = run_bass_kernel_spmd(nc, in_maps, core_ids=list(range(NCORES)))
    _cache["last_res"] = res
    out = np.empty((B, NCLS), np.float32)
    for core in range(NCORES):
        out[core * BPC:(core + 1) * BPC] = res.results[core]["logitsT"].T
    return out


# revision 21
# speedup vs baseline: 1.1929x; 1.1929x over previous
"""Trainium2 Bass kernel for BertWithAdaThresholdLocContextPooling.

Strategy: pure data parallel over batch (B=16 -> 2 batches per core x 8 cores).
Each core:
  - gathers mention rows of sequence_output / attention via indirect DMA
    (only ~0.2MB of the 12.6MB attention shard is ever read from HBM),
  - logsumexp-pools mention embeddings, mean-pools attention rows,
  - computes the localized-context attention rs = seq^T @ ht,
  - runs the two extractor GEMVs (bf16 data, fp32 accumulate),
  - forms the grouped bilinear via PE replication matmuls,
  - applies the classifier Wb.
Weights are replicated to all cores; the host pre-transposes/casts them and
packs small constants so each core issues only a handful of large DMAs.
"""

import sys

for _p in ("/opt/trn_rl_repo",):
    if _p not in sys.path:
        sys.path.insert(0, _p)

import numpy as np
import ml_dtypes

import concourse.bacc as bacc
import concourse.bass as bass
import concourse.mybir as mybir
from concourse.tile import TileContext
from concourse.bass_utils import run_bass_kernel_spmd

F32 = mybir.dt.float32
BF16 = mybir.dt.bfloat16
I32 = mybir.dt.int32
AF = mybir.ActivationFunctionType
ALU = mybir.AluOpType

B, L, HID = 16, 512, 768
HEADS, M = 12, 4
EMB, BLK, NER, NCLS = 768, 8, 6, 97
NCORES = 8
BPC = B // NCORES          # batches per core = 2
CAT = 2 * HID + NER        # 1542
KCH = 12                   # full 128-row contraction chunks of CAT
NEMB = EMB // 128          # 6 chunks of EMB
NL = L // 128              # 4 chunks of L
NBL = EMB * BLK // 128     # 48 classifier contraction chunks

# packed-constant layouts
# CIDX [96, 99] f32 (critical path): rep8 [0:8,0:96] | baseA [0:96,96:98]
#                                    | baseS [0:16,98:99]
CIDX_COLS = 99
# CID2 [128, 129] f32: identity [0:128,0:128] | bbc [0:97,128:129]
CID2_COLS = 129
# CBA [96, 29] bf16 (early): selE [0:16,0:4] | selA [0:96,4:28] | w12 [0:12,28:29]
CBA_COLS = 29
# CBB [128, 2568] bf16 (late): rys [0:128,0:1024] | bhr [0:1,1024:1792]
#   | btr [0:1,1792:2560] | selbh [0:1,2560:2564] | selbt [0:1,2564:2568]
RYS0 = 0
BHR0 = 1024
BTR0 = BHR0 + 768
SELBH0 = BTR0 + 768
CBB_COLS = SELBH0 + 8

_cache = {}


def _build_constants():
    selE = np.zeros((4 * M, 4), np.float32)
    for k in range(4 * M):
        selE[k, k // M] = 1.0
    selA = np.zeros((2 * M * HEADS, 2 * HEADS), np.float32)
    for i in range(2):
        for m in range(M):
            for h in range(HEADS):
                selA[i * M * HEADS + m * HEADS + h, i * HEADS + h] = 1.0 / M
    rep8 = np.zeros((2 * M, 2 * M * HEADS), np.float32)
    for q in range(2 * M * HEADS):
        rep8[q // HEADS, q] = 1.0
    baseA = np.zeros((2 * M * HEADS, BPC), np.float32)
    for q in range(2 * M * HEADS):
        for b in range(BPC):
            baseA[q, b] = (b * HEADS + q % HEADS) * L + 1
    baseS = np.zeros((4 * M, 1), np.float32)
    for k in range(4 * M):
        baseS[k, 0] = (k // (2 * M)) * L + 1

    cidx = np.zeros((96, CIDX_COLS), np.float32)
    cidx[0:8, 0:96] = rep8
    cidx[0:96, 96:98] = baseA
    cidx[0:16, 98:99] = baseS
    cid2 = np.zeros((128, CID2_COLS), np.float32)
    cid2[0:128, 0:128] = np.eye(128)
    # bbc filled per-call (bias input)

    cba = np.zeros((96, CBA_COLS), ml_dtypes.bfloat16)
    cba[0:16, 0:4] = selE
    cba[0:96, 4:28] = selA
    cba[0:12, 28:29] = 1.0 / HEADS
    cbb = np.zeros((128, CBB_COLS), ml_dtypes.bfloat16)
    for y in range(BLK):
        for p in range(128):
            cbb[(p // BLK) * BLK + y, RYS0 + y * 128 + p] = 1.0
    cbb[0:1, SELBH0:SELBH0 + 4] = np.array([1.0, 0.0, 1.0, 0.0])
    cbb[0:1, SELBH0 + 4:SELBH0 + 8] = np.array([0.0, 1.0, 0.0, 1.0])

    perm = np.empty(EMB * BLK, np.int64)
    for cch in range(NEMB):
        for y in range(BLK):
            for p in range(128):
                g = cch * 16 + p // BLK
                x = p % BLK
                perm[(cch * BLK + y) * 128 + p] = g * 64 + x * BLK + y
    return {"cidx": cidx, "cid2": cid2, "cba": cba, "cbb": cbb, "perm": perm}


def _build_program(stage=99):
    nc = bacc.Bacc("TRN2", target_bir_lowering=False, debug=False)

    seq_h = nc.dram_tensor("seq", [BPC * L, HID], BF16, kind="ExternalInput")
    attn_h = nc.dram_tensor("attn", [BPC * HEADS * L, L], BF16, kind="ExternalInput")
    pos_h = nc.dram_tensor("pos", [4 * M, 1], I32, kind="ExternalInput")
    posb_hs = [
        nc.dram_tensor(f"posb{b}", [2 * M, 1], I32, kind="ExternalInput")
        for b in range(BPC)
    ]
    ner_h = nc.dram_tensor("ner", [NER, 4], F32, kind="ExternalInput")
    whs_h = nc.dram_tensor("whs", [128, KCH * EMB + EMB], BF16, kind="ExternalInput")
    wts_h = nc.dram_tensor("wts", [128, KCH * EMB + EMB], BF16, kind="ExternalInput")
    wbs_h = nc.dram_tensor("wbs", [128, NBL * 128], BF16, kind="ExternalInput")
    cidx_h = nc.dram_tensor("cidx", [96, CIDX_COLS], F32, kind="ExternalInput")
    cid2_h = nc.dram_tensor("cid2", [128, CID2_COLS], F32, kind="ExternalInput")
    cba_h = nc.dram_tensor("cba", [96, CBA_COLS], BF16, kind="ExternalInput")
    cbb_h = nc.dram_tensor("cbb", [128, CBB_COLS], BF16, kind="ExternalInput")
    out_h = nc.dram_tensor("logitsT", [NCLS, BPC], F32, kind="ExternalOutput")

    with TileContext(nc) as tc:
        with (
            tc.tile_pool(name="const", bufs=1) as cp,
            tc.tile_pool(name="data", bufs=1) as dp,
            tc.tile_pool(name="psbig", bufs=1, space="PSUM") as psb,
            tc.tile_pool(name="psea", bufs=2, space="PSUM") as pse,
            tc.tile_pool(name="pssm", bufs=3, space="PSUM") as pss,
        ):
            # ---- critical small loads first (sync queue) ----
            cidx = cp.tile([96, CIDX_COLS], F32)
            nc.sync.dma_start(cidx[:], cidx_h[:])
            posi = dp.tile([4 * M, 1], I32)
            pos_dma = nc.sync.dma_start(posi[:], pos_h[:])
            posbi = []
            for b in range(BPC):
                t = dp.tile([2 * M, 1], I32, tag=f"posbi{b}")
                nc.sync.dma_start(t[:], posb_hs[b][:])
                posbi.append(t)
            cba = cp.tile([96, CBA_COLS], BF16)
            nc.sync.dma_start(cba[:], cba_h[:])
            cid2 = cp.tile([128, CID2_COLS], F32)
            nc.sync.dma_start(cid2[:], cid2_h[:])
            rep8 = cidx[0:8, 0:96]
            baseA = cidx[0:96, 96:98]
            baseS = cidx[0:16, 98:99]
            bbc = cid2[0:97, 128:129]
            selE = cba[0:16, 0:4]
            selA = cba[0:96, 4:28]
            w12 = cba[0:12, 28:29]
            idf = cid2[:, 0:128]

            # ---- index computation ----
            posf = dp.tile([4 * M, 1], F32)
            nc.vector.tensor_copy(posf[:], posi[:])
            idxsf = dp.tile([4 * M, 1], F32)
            nc.vector.tensor_add(idxsf[:], posf[:], baseS)
            idxs = dp.tile([4 * M, 1], I32)
            nc.vector.tensor_copy(idxs[:], idxsf[:])

            idxa = []
            for b in range(BPC):
                posbf = dp.tile([2 * M, 1], F32, tag=f"posbf{b}")
                nc.vector.tensor_copy(posbf[:], posbi[b][:])
                ps_idx = pss.tile([2 * M * HEADS, 1], F32, tag="sm")
                nc.tensor.matmul(ps_idx[:], lhsT=rep8, rhs=posbf[:],
                                 start=True, stop=True)
                idxaf = dp.tile([2 * M * HEADS, 1], F32, tag=f"idxaf{b}")
                nc.vector.tensor_add(idxaf[:], ps_idx[:], baseA[:, b:b + 1])
                ia = dp.tile([2 * M * HEADS, 1], I32, tag=f"idxa{b}")
                nc.vector.tensor_copy(ia[:], idxaf[:])
                idxa.append(ia)

            # ---- gathers ----
            sg = dp.tile([4 * M, HID], BF16)
            g0 = nc.gpsimd.indirect_dma_start(
                out=sg[:], out_offset=None, in_=seq_h[:],
                in_offset=bass.IndirectOffsetOnAxis(ap=idxs[:, :1], axis=0))
            at = []
            for b in range(BPC):
                t = dp.tile([2 * M * HEADS, L], BF16, tag=f"at{b}")
                g = nc.gpsimd.indirect_dma_start(
                    out=t[:], out_offset=None, in_=attn_h[:],
                    in_offset=bass.IndirectOffsetOnAxis(ap=idxa[b][:, :1], axis=0))
                at.append(t)

            # ---- bulk loads, held behind the gathers so the tiny critical
            # transfers are not starved of SDMA bandwidth ----
            from concourse.tile_rust import add_dep_helper
            big = []
            whsf = cp.tile([128, KCH * EMB + EMB], BF16)
            big.append(nc.scalar.dma_start(whsf[:], whs_h[:]))
            whs = whsf[:, 0:KCH * EMB]
            whn = whsf[0:NER, KCH * EMB:KCH * EMB + EMB]
            wtsf = cp.tile([128, KCH * EMB + EMB], BF16)
            big.append(nc.scalar.dma_start(wtsf[:], wts_h[:]))
            wts = wtsf[:, 0:KCH * EMB]
            wtn = wtsf[0:NER, KCH * EMB:KCH * EMB + EMB]
            seqt = []
            for b in range(BPC):
                t = dp.tile([128, NL * HID], BF16, tag=f"seq{b}")
                big.append(nc.sync.dma_start(
                    t[:].rearrange("p (c d) -> p c d", c=NL),
                    seq_h[b * L:(b + 1) * L, :].rearrange("(c p) d -> p c d", p=128)))
                seqt.append(t)
            wbs = cp.tile([128, NBL * 128], BF16)
            big.append(nc.scalar.dma_start(wbs[:], wbs_h[:]))
            cbb = cp.tile([128, CBB_COLS], BF16)
            big.append(nc.sync.dma_start(cbb[:], cbb_h[:]))
            rys = cbb[:, RYS0:RYS0 + 1024]
            bhr = cbb[0:1, BHR0:BHR0 + EMB]
            btr = cbb[0:1, BTR0:BTR0 + EMB]
            selbh = cbb[0:1, SELBH0:SELBH0 + 4]
            selbt = cbb[0:1, SELBH0 + 4:SELBH0 + 8]
            ner4f = dp.tile([NER, 4], F32)
            big.append(nc.sync.dma_start(ner4f[:], ner_h[:]))
            ner4 = dp.tile([NER, 4], BF16)
            nc.vector.tensor_copy(ner4[:], ner4f[:])
            for d in big:
                add_dep_helper(d.ins, pos_dma.ins,
                               reason="bulk loads yield SDMA to index loads")

            if stage < 1:
                lg = dp.tile([NCLS, BPC], F32)
                nc.vector.memset(lg[:], 0.0)
                nc.vector.tensor_copy(lg[0:4 * M, 0:1], sg[:, 0:1])
                nc.vector.tensor_copy(lg[0:2 * M * HEADS - 31, 1:2], at[0][0:2 * M * HEADS - 31, 0:1])
                nc.sync.dma_start(out_h[:], lg[:])
                return _finish(nc)

            # ---- entity embeddings: log-sum-exp over mentions ----
            exps = dp.tile([4 * M, HID], BF16)
            nc.scalar.activation(exps[:], sg[:], AF.Exp)
            ps_e = psb.tile([4, HID], F32, tag="big")
            for n0, nl_ in ((0, 512), (512, 256)):
                nc.tensor.matmul(ps_e[:, n0:n0 + nl_], lhsT=selE,
                                 rhs=exps[:, n0:n0 + nl_], start=True, stop=True)
            ent = dp.tile([4, HID], F32)
            nc.scalar.activation(ent[:], ps_e[:], AF.Ln)
            ps_et = pss.tile([128, 4 * NEMB], F32, tag="sm")
            for c in range(NEMB):
                nc.tensor.transpose(ps_et[:, c * 4:(c + 1) * 4],
                                    ent[:, c * 128:(c + 1) * 128], idf[0:4, 0:4])
            entT = dp.tile([128, 4 * NEMB], BF16)
            nc.vector.tensor_copy(entT[:], ps_et[:])

            # ---- entity attention pooling + context vector ----
            htc = []
            for b in range(BPC):
                ps_eah = pse.tile([HEADS, L], F32, tag="ea")
                nc.tensor.matmul(ps_eah[:], lhsT=selA[:, 0:HEADS], rhs=at[b][:],
                                 start=True, stop=True)
                ps_eat = pse.tile([HEADS, L], F32, tag="ea")
                nc.tensor.matmul(ps_eat[:], lhsT=selA[:, HEADS:2 * HEADS],
                                 rhs=at[b][:], start=True, stop=True)
                eah = dp.tile([HEADS, L], F32, tag=f"eah{b}")
                nc.vector.tensor_copy(eah[:], ps_eah[:])
                prd = dp.tile([HEADS, L], BF16, tag=f"prd{b}")
                nc.vector.tensor_tensor(out=prd[:], in0=eah[:], in1=ps_eat[:],
                                        op=ALU.mult)
                ps_ht = pss.tile([1, L], F32, tag="sm")
                nc.tensor.matmul(ps_ht[:], lhsT=w12, rhs=prd[:],
                                 start=True, stop=True)
                sm = dp.tile([1, 1], F32, tag=f"sm{b}")
                nc.vector.reduce_sum(sm[:], ps_ht[:], axis=mybir.AxisListType.X)
                den = dp.tile([1, 1], F32, tag=f"den{b}")
                nc.vector.tensor_scalar_add(den[:], sm[:], 1e-5)
                rcp = dp.tile([1, 1], F32, tag=f"rcp{b}")
                nc.vector.reciprocal(rcp[:], den[:])
                htn = dp.tile([1, L], F32, tag=f"htn{b}")
                nc.vector.tensor_scalar_mul(htn[:], ps_ht[:], rcp[:, :1])
                ps_htc = pss.tile([128, NL], F32, tag="sm")
                for c in range(NL):
                    nc.tensor.transpose(ps_htc[:, c:c + 1],
                                        htn[:, c * 128:(c + 1) * 128],
                                        idf[0:1, 0:1])
                h = dp.tile([128, NL], BF16, tag=f"htc{b}")
                nc.vector.tensor_copy(h[:], ps_htc[:])
                htc.append(h)

            if stage < 2:
                lg = dp.tile([NCLS, BPC], F32)
                nc.vector.memset(lg[:], 0.0)
                nc.vector.tensor_copy(lg[0:97, 0:1], entT[0:97, 0:1])
                nc.vector.tensor_copy(lg[0:97, 1:2], htc[0][0:97, 0:1])
                nc.sync.dma_start(out_h[:], lg[:])
                return _finish(nc)

            # ---- rs = seq^T @ ht  (column form) ----
            ps_rsc = pss.tile([128, NEMB * BPC], F32, tag="sm")
            for b in range(BPC):
                for d in range(NEMB):
                    for c in range(NL):
                        nc.tensor.matmul(
                            ps_rsc[:, d * BPC + b:d * BPC + b + 1],
                            lhsT=seqt[b][:, c * HID + d * 128:c * HID + (d + 1) * 128],
                            rhs=htc[b][:, c:c + 1],
                            start=(c == 0), stop=(c == NL - 1))
            rsc = dp.tile([128, 4 * NEMB], BF16)
            nc.vector.tensor_copy(
                rsc[:].rearrange("p (r b m) -> p r b m", r=NEMB, b=BPC),
                ps_rsc[:].rearrange("p (r b) -> p r b", r=NEMB)
                .unsqueeze(3).broadcast_to([128, NEMB, BPC, 2]))

            # ---- extractor GEMVs:  [4,768] = cat4^T @ W^T  ----
            def cat_chunk(j):
                if j < NEMB:
                    return entT[:, j * 4:(j + 1) * 4]
                if j < 2 * NEMB:
                    return rsc[:, (j - NEMB) * 4:(j - NEMB + 1) * 4]
                return ner4[:]

            t4 = []
            for wi, (ws, wn, selb, br) in enumerate(
                    ((whs, whn, selbh, bhr), (wts, wtn, selbt, btr))):
                ps_w = psb.tile([4, EMB], F32, tag="big")
                for n0, nl_ in ((0, 512), (512, 256)):
                    for j in range(KCH + 1):
                        lhsT = cat_chunk(j)
                        rhs = (ws[:, j * EMB + n0:j * EMB + n0 + nl_] if j < KCH
                               else wn[:, n0:n0 + nl_])
                        nc.tensor.matmul(ps_w[:, n0:n0 + nl_], lhsT=lhsT, rhs=rhs,
                                         start=(j == 0), stop=False)
                    nc.tensor.matmul(ps_w[:, n0:n0 + nl_], lhsT=selb,
                                     rhs=br[:, n0:n0 + nl_], start=False, stop=True)
                t = dp.tile([4, EMB], F32, tag=f"t4_{wi}")
                nc.scalar.activation(t[:], ps_w[:], AF.Tanh)
                t4.append(t)

            if stage < 3:
                lg = dp.tile([NCLS, BPC], F32)
                nc.vector.memset(lg[:], 0.0)
                nc.vector.tensor_copy(lg[0:4, 0:2], t4[0][:, 0:2])
                nc.vector.tensor_copy(lg[0:4, 1:2], t4[1][:, 0:1])
                nc.vector.tensor_copy(lg[0:89, 0:1], rsc[0:89, 0:1])
                nc.sync.dma_start(out_h[:], lg[:])
                return _finish(nc)

            # ---- transpose hs2/ts2 to columns ----
            ps_a = pss.tile([128, 4 * NEMB], F32, tag="sm")
            ps_b2 = pss.tile([128, 4 * NEMB], F32, tag="sm")
            for c in range(NEMB):
                nc.tensor.transpose(ps_a[:, c * 4:(c + 1) * 4],
                                    t4[0][:, c * 128:(c + 1) * 128], idf[0:4, 0:4])
                nc.tensor.transpose(ps_b2[:, c * 4:(c + 1) * 4],
                                    t4[1][:, c * 128:(c + 1) * 128], idf[0:4, 0:4])
            h2t = dp.tile([128, 4 * NEMB], BF16)
            nc.vector.tensor_copy(
                h2t[:].rearrange("p (c b) -> p c b", c=NEMB)[:, :, 0:4:2],
                ps_a[:].rearrange("p (c b) -> p c b", c=NEMB)[:, :, 0:4:2])
            nc.vector.tensor_copy(
                h2t[:].rearrange("p (c b) -> p c b", c=NEMB)[:, :, 1:4:2],
                ps_b2[:].rearrange("p (c b) -> p c b", c=NEMB)[:, :, 1:4:2])

            if stage < 4:
                lg = dp.tile([NCLS, BPC], F32)
                nc.vector.memset(lg[:], 0.0)
                nc.vector.tensor_copy(lg[0:97, 0:2], h2t[0:97, 0:2])
                nc.sync.dma_start(out_h[:], lg[:])
                return _finish(nc)

            # ---- grouped bilinear + classifier ----
            # ts-replication: out col layout (y, c, b) = y*12 + c*2 + b
            ps_t2x = pss.tile([128, BLK * NEMB * BPC], F32, tag="sm")
            tscols = h2t[:].rearrange("p (c b) -> p c b", c=NEMB)[:, :, 1:4:2]
            for y in range(BLK):
                nc.tensor.matmul(
                    ps_t2x[:, y * 12:(y + 1) * 12]
                    .rearrange("p (c b) -> p c b", c=NEMB),
                    lhsT=rys[:, y * 128:(y + 1) * 128],
                    rhs=tscols, start=True, stop=True)
            if stage < 5:
                lg = dp.tile([NCLS, BPC], F32)
                nc.vector.memset(lg[:], 0.0)
                nc.vector.tensor_copy(lg[0:97, 0:2], ps_t2x[0:97, 0:2])
                nc.sync.dma_start(out_h[:], lg[:])
                return _finish(nc)

            blt = dp.tile([128, NEMB * 16], BF16)
            for c in range(NEMB):
                nc.vector.tensor_tensor(
                    out=blt[:, c * 16:(c + 1) * 16]
                    .rearrange("p (y b) -> p y b", y=BLK),
                    in0=h2t[:, c * 4:c * 4 + 4:2].unsqueeze(1)
                        .broadcast_to([128, BLK, 2]),
                    in1=ps_t2x[:].rearrange("p (y c b) -> p y c b", y=BLK, c=NEMB)
                    [:, :, c, :],
                    op=ALU.mult)
            if stage < 6:
                lg = dp.tile([NCLS, BPC], F32)
                nc.vector.memset(lg[:], 0.0)
                nc.vector.tensor_copy(lg[0:97, 0:2], blt[0:97, 0:2])
                nc.sync.dma_start(out_h[:], lg[:])
                return _finish(nc)

            ps_l = pss.tile([NCLS, BPC], F32, tag="sm")
            for c in range(NEMB):
                for y in range(BLK):
                    k = c * BLK + y
                    nc.tensor.matmul(ps_l[:], lhsT=wbs[:, k * 128:k * 128 + NCLS],
                                     rhs=blt[:, c * 16 + y * 2:c * 16 + y * 2 + 2],
                                     start=(k == 0), stop=(k == NBL - 1))
            lg = dp.tile([NCLS, BPC], F32)
            if stage < 7:
                nc.vector.memset(lg[:], 0.0)
                nc.vector.tensor_copy(lg[0:1, 0:1], ps_l[0:1, 0:1])
            else:
                nc.vector.tensor_scalar_add(lg[:], ps_l[:], bbc[:, :1])
            nc.sync.dma_start(out_h[:], lg[:])

    return _finish(nc)


def _finish(nc):
    return nc


def _get_program():
    if "nc" not in _cache:
        nc = _build_program()
        nc.finalize()
        _cache["nc"] = nc
        _cache["consts"] = _build_constants()
    return _cache["nc"], _cache["consts"]


def kernel(sequence_output, attention, entity_pos, hs_ner_tags, ts_ner_tags,
           Wh, bh, Wt, bt, Wb, bb):
    nc, c = _get_program()

    seq = np.asarray(sequence_output, dtype=np.float32).astype(ml_dtypes.bfloat16)
    attn = np.asarray(attention, dtype=np.float32).astype(ml_dtypes.bfloat16)
    pos = np.asarray(entity_pos).astype(np.int32)
    nh = np.asarray(hs_ner_tags, dtype=np.float32)
    nt = np.asarray(ts_ner_tags, dtype=np.float32)
    whT = np.ascontiguousarray(np.asarray(Wh, dtype=np.float32).T).astype(ml_dtypes.bfloat16)
    wtT = np.ascontiguousarray(np.asarray(Wt, dtype=np.float32).T).astype(ml_dtypes.bfloat16)
    wbT = np.ascontiguousarray(np.asarray(Wb, dtype=np.float32).T)[c["perm"]]
    wbT = np.pad(wbT, ((0, 0), (0, 128 - NCLS))).astype(ml_dtypes.bfloat16)

    def sbuf_image(w, extra):
        main = w[0:KCH * 128].reshape(KCH, 128, EMB).transpose(1, 0, 2).reshape(128, KCH * EMB)
        img = np.zeros((128, KCH * EMB + EMB), ml_dtypes.bfloat16)
        img[:, 0:KCH * EMB] = main
        img[0:NER, KCH * EMB:] = extra
        return img

    whs = sbuf_image(whT, whT[KCH * 128:CAT])
    wts = sbuf_image(wtT, wtT[KCH * 128:CAT])
    wbs = wbT.reshape(NBL, 128, 128).transpose(1, 0, 2).reshape(128, NBL * 128)
    wbs = np.ascontiguousarray(wbs)

    c["cbb"][0:1, BHR0:BHR0 + EMB] = np.asarray(bh, np.float32).reshape(1, EMB)
    c["cbb"][0:1, BTR0:BTR0 + EMB] = np.asarray(bt, np.float32).reshape(1, EMB)
    cid2 = c["cid2"].copy()
    cid2[0:97, 128] = np.asarray(bb, np.float32)

    in_maps = []
    for core in range(NCORES):
        b0 = core * BPC
        pc = np.ascontiguousarray(pos[b0:b0 + BPC])          # [2,2,M]
        ner = np.stack([nh[b0], nt[b0], nh[b0 + 1], nt[b0 + 1]], axis=1)
        im = {
            "seq": np.ascontiguousarray(seq[b0:b0 + BPC]).reshape(BPC * L, HID),
            "attn": np.ascontiguousarray(attn[b0:b0 + BPC]).reshape(BPC * HEADS * L, L),
            "pos": pc.reshape(4 * M, 1),
            "ner": np.ascontiguousarray(ner.astype(np.float32)),
            "whs": whs, "wts": wts, "wbs": wbs,
            "cidx": c["cidx"], "cid2": cid2, "cba": c["cba"], "cbb": c["cbb"],
        }
        for b in range(BPC):
            im[f"posb{b}"] = np.ascontiguousarray(pc[b].reshape(2 * M, 1))
        in_maps.append(im)

    res = run_bass_kernel_spmd(nc, in_maps, core_ids=list(range(NCORES)))
    _cache["last_res"] = res
    out = np.empty((B, NCLS), np.float32)
    for core in range(NCORES):
        out[core * BPC:(core + 1) * BPC] = res.results[core]["logitsT"].T
    return out


# revision 22
# speedup vs baseline: 1.2334x; 1.0339x over previous
"""Trainium2 Bass kernel for BertWithAdaThresholdLocContextPooling.

Strategy: pure data parallel over batch (B=16 -> 2 batches per core x 8 cores).
Each core:
  - gathers mention rows of sequence_output / attention via indirect DMA
    (only ~0.2MB of the 12.6MB attention shard is ever read from HBM),
  - logsumexp-pools mention embeddings, mean-pools attention rows,
  - computes the localized-context attention rs = seq^T @ ht,
  - runs the two extractor GEMVs (bf16 data, fp32 accumulate),
  - forms the grouped bilinear via PE replication matmuls,
  - applies the classifier Wb.
Weights are replicated to all cores; the host pre-transposes/casts them and
packs small constants so each core issues only a handful of large DMAs.
"""

import sys

for _p in ("/opt/trn_rl_repo",):
    if _p not in sys.path:
        sys.path.insert(0, _p)

import numpy as np
import ml_dtypes

import concourse.bacc as bacc
import concourse.bass as bass
import concourse.mybir as mybir
from concourse.tile import TileContext
from concourse.bass_utils import run_bass_kernel_spmd

F32 = mybir.dt.float32
BF16 = mybir.dt.bfloat16
I32 = mybir.dt.int32
AF = mybir.ActivationFunctionType
ALU = mybir.AluOpType

B, L, HID = 16, 512, 768
HEADS, M = 12, 4
EMB, BLK, NER, NCLS = 768, 8, 6, 97
NCORES = 8
BPC = B // NCORES          # batches per core = 2
CAT = 2 * HID + NER        # 1542
KCH = 12                   # full 128-row contraction chunks of CAT
NEMB = EMB // 128          # 6 chunks of EMB
NL = L // 128              # 4 chunks of L
NBL = EMB * BLK // 128     # 48 classifier contraction chunks

# packed-constant layouts
# CIDX [96, 99] f32 (critical path): rep8 [0:8,0:96] | baseA [0:96,96:98]
#                                    | baseS [0:16,98:99]
CIDX_COLS = 99
# CID2 [128, 129] f32: identity [0:128,0:128] | bbc [0:97,128:129]
CID2_COLS = 129
# CBA [96, 29] bf16 (early): selE [0:16,0:4] | selA [0:96,4:28] | w12 [0:12,28:29]
CBA_COLS = 29
# CBB [128, 2568] bf16 (late): rys [0:128,0:1024] | bhr [0:1,1024:1792]
#   | btr [0:1,1792:2560] | selbh [0:1,2560:2564] | selbt [0:1,2564:2568]
RYS0 = 0
BHR0 = 1024
BTR0 = BHR0 + 768
SELBH0 = BTR0 + 768
CBB_COLS = SELBH0 + 8

_cache = {}


def _build_constants():
    selE = np.zeros((4 * M, 4), np.float32)
    for k in range(4 * M):
        selE[k, k // M] = 1.0
    selA = np.zeros((2 * M * HEADS, 2 * HEADS), np.float32)
    for i in range(2):
        for m in range(M):
            for h in range(HEADS):
                selA[i * M * HEADS + m * HEADS + h, i * HEADS + h] = 1.0 / M
    rep8 = np.zeros((2 * M, 2 * M * HEADS), np.float32)
    for q in range(2 * M * HEADS):
        rep8[q // HEADS, q] = 1.0
    baseA = np.zeros((2 * M * HEADS, BPC), np.float32)
    for q in range(2 * M * HEADS):
        for b in range(BPC):
            baseA[q, b] = (b * HEADS + q % HEADS) * L + 1
    baseS = np.zeros((4 * M, 1), np.float32)
    for k in range(4 * M):
        baseS[k, 0] = (k // (2 * M)) * L + 1

    cidx = np.zeros((96, CIDX_COLS), np.float32)
    cidx[0:8, 0:96] = rep8
    cidx[0:96, 96:98] = baseA
    cidx[0:16, 98:99] = baseS
    cid2 = np.zeros((128, CID2_COLS), np.float32)
    cid2[0:128, 0:128] = np.eye(128)
    # bbc filled per-call (bias input)

    cba = np.zeros((96, CBA_COLS), ml_dtypes.bfloat16)
    cba[0:16, 0:4] = selE
    cba[0:96, 4:28] = selA
    cba[0:12, 28:29] = 1.0 / HEADS
    cbb = np.zeros((128, CBB_COLS), ml_dtypes.bfloat16)
    for y in range(BLK):
        for p in range(128):
            cbb[(p // BLK) * BLK + y, RYS0 + y * 128 + p] = 1.0
    cbb[0:1, SELBH0:SELBH0 + 4] = np.array([1.0, 0.0, 1.0, 0.0])
    cbb[0:1, SELBH0 + 4:SELBH0 + 8] = np.array([0.0, 1.0, 0.0, 1.0])

    perm = np.empty(EMB * BLK, np.int64)
    for cch in range(NEMB):
        for y in range(BLK):
            for p in range(128):
                g = cch * 16 + p // BLK
                x = p % BLK
                perm[(cch * BLK + y) * 128 + p] = g * 64 + x * BLK + y
    return {"cidx": cidx, "cid2": cid2, "cba": cba, "cbb": cbb, "perm": perm}


def _build_program(stage=99):
    nc = bacc.Bacc("TRN2", target_bir_lowering=False, debug=False)

    seq_h = nc.dram_tensor("seq", [BPC * L, HID], BF16, kind="ExternalInput")
    attn_h = nc.dram_tensor("attn", [BPC * HEADS * L, L], BF16, kind="ExternalInput")
    pos_h = nc.dram_tensor("pos", [4 * M, 1], I32, kind="ExternalInput")
    posb_hs = [
        nc.dram_tensor(f"posb{b}", [2 * M, 1], I32, kind="ExternalInput")
        for b in range(BPC)
    ]
    ner_h = nc.dram_tensor("ner", [NER, 4], F32, kind="ExternalInput")
    whs_h = nc.dram_tensor("whs", [128, KCH * EMB + EMB], BF16, kind="ExternalInput")
    wts_h = nc.dram_tensor("wts", [128, KCH * EMB + EMB], BF16, kind="ExternalInput")
    wbs_h = nc.dram_tensor("wbs", [128, NBL * 128], BF16, kind="ExternalInput")
    cidx_h = nc.dram_tensor("cidx", [96, CIDX_COLS], F32, kind="ExternalInput")
    cid2_h = nc.dram_tensor("cid2", [128, CID2_COLS], F32, kind="ExternalInput")
    cba_h = nc.dram_tensor("cba", [96, CBA_COLS], BF16, kind="ExternalInput")
    cbb_h = nc.dram_tensor("cbb", [128, CBB_COLS], BF16, kind="ExternalInput")
    out_h = nc.dram_tensor("logitsT", [NCLS, BPC], F32, kind="ExternalOutput")

    with TileContext(nc) as tc:
        with (
            tc.tile_pool(name="const", bufs=1) as cp,
            tc.tile_pool(name="data", bufs=1) as dp,
            tc.tile_pool(name="psbig", bufs=1, space="PSUM") as psb,
            tc.tile_pool(name="psea", bufs=2, space="PSUM") as pse,
            tc.tile_pool(name="pssm", bufs=3, space="PSUM") as pss,
        ):
            # ---- critical small loads first (sync queue) ----
            cidx = cp.tile([96, CIDX_COLS], F32)
            nc.sync.dma_start(cidx[:], cidx_h[:])
            posi = dp.tile([4 * M, 1], I32)
            pos_dma = nc.sync.dma_start(posi[:], pos_h[:])
            posbi = []
            for b in range(BPC):
                t = dp.tile([2 * M, 1], I32, tag=f"posbi{b}")
                nc.sync.dma_start(t[:], posb_hs[b][:])
                posbi.append(t)
            cba = cp.tile([96, CBA_COLS], BF16)
            nc.sync.dma_start(cba[:], cba_h[:])
            cid2 = cp.tile([128, CID2_COLS], F32)
            nc.sync.dma_start(cid2[:], cid2_h[:])
            rep8 = cidx[0:8, 0:96]
            baseA = cidx[0:96, 96:98]
            baseS = cidx[0:16, 98:99]
            bbc = cid2[0:97, 128:129]
            selE = cba[0:16, 0:4]
            selA = cba[0:96, 4:28]
            w12 = cba[0:12, 28:29]
            idf = cid2[:, 0:128]

            # ---- index computation ----
            posf = dp.tile([4 * M, 1], F32)
            nc.vector.tensor_copy(posf[:], posi[:])
            idxsf = dp.tile([4 * M, 1], F32)
            nc.vector.tensor_add(idxsf[:], posf[:], baseS)
            idxs = dp.tile([4 * M, 1], I32)
            nc.vector.tensor_copy(idxs[:], idxsf[:])

            idxa = []
            for b in range(BPC):
                posbf = dp.tile([2 * M, 1], F32, tag=f"posbf{b}")
                nc.vector.tensor_copy(posbf[:], posbi[b][:])
                ps_idx = pss.tile([2 * M * HEADS, 1], F32, tag="sm")
                nc.tensor.matmul(ps_idx[:], lhsT=rep8, rhs=posbf[:],
                                 start=True, stop=True)
                idxaf = dp.tile([2 * M * HEADS, 1], F32, tag=f"idxaf{b}")
                nc.vector.tensor_add(idxaf[:], ps_idx[:], baseA[:, b:b + 1])
                ia = dp.tile([2 * M * HEADS, 1], I32, tag=f"idxa{b}")
                nc.vector.tensor_copy(ia[:], idxaf[:])
                idxa.append(ia)

            # ---- gathers ----
            sg = dp.tile([4 * M, HID], BF16)
            g0 = nc.gpsimd.indirect_dma_start(
                out=sg[:], out_offset=None, in_=seq_h[:],
                in_offset=bass.IndirectOffsetOnAxis(ap=idxs[:, :1], axis=0))
            at = []
            for b in range(BPC):
                t = dp.tile([2 * M * HEADS, L], BF16, tag=f"at{b}")
                g = nc.gpsimd.indirect_dma_start(
                    out=t[:], out_offset=None, in_=attn_h[:],
                    in_offset=bass.IndirectOffsetOnAxis(ap=idxa[b][:, :1], axis=0))
                at.append(t)

            # ---- bulk loads, held behind the gathers so the tiny critical
            # transfers are not starved of SDMA bandwidth ----
            from concourse.tile_rust import add_dep_helper
            big = []
            whsf = cp.tile([128, KCH * EMB + EMB], BF16)
            big.append(nc.scalar.dma_start(whsf[:], whs_h[:]))
            whs = whsf[:, 0:KCH * EMB]
            whn = whsf[0:NER, KCH * EMB:KCH * EMB + EMB]
            wtsf = cp.tile([128, KCH * EMB + EMB], BF16)
            big.append(nc.scalar.dma_start(wtsf[:], wts_h[:]))
            wts = wtsf[:, 0:KCH * EMB]
            wtn = wtsf[0:NER, KCH * EMB:KCH * EMB + EMB]
            seqt = []
            for b in range(BPC):
                t = dp.tile([128, NL * HID], BF16, tag=f"seq{b}")
                big.append(nc.sync.dma_start(
                    t[:].rearrange("p (c d) -> p c d", c=NL),
                    seq_h[b * L:(b + 1) * L, :].rearrange("(c p) d -> p c d", p=128)))
                seqt.append(t)
            wbs = cp.tile([128, NBL * 128], BF16)
            big.append(nc.scalar.dma_start(wbs[:], wbs_h[:]))
            cbb = cp.tile([128, CBB_COLS], BF16)
            big.append(nc.sync.dma_start(cbb[:], cbb_h[:]))
            rys = cbb[:, RYS0:RYS0 + 1024]
            bhr = cbb[0:1, BHR0:BHR0 + EMB]
            btr = cbb[0:1, BTR0:BTR0 + EMB]
            selbh = cbb[0:1, SELBH0:SELBH0 + 4]
            selbt = cbb[0:1, SELBH0 + 4:SELBH0 + 8]
            ner4f = dp.tile([NER, 4], F32)
            big.append(nc.sync.dma_start(ner4f[:], ner_h[:]))
            ner4 = dp.tile([NER, 4], BF16)
            nc.vector.tensor_copy(ner4[:], ner4f[:])
            for d in big:
                add_dep_helper(d.ins, pos_dma.ins,
                               reason="bulk loads yield SDMA to index loads")
            # chain the bulk transfers so early-needed ones get full bandwidth
            for a, b2 in ((big[1], big[0]), (big[3], big[2]),
                          (big[4], big[1]), (big[5], big[4])):
                add_dep_helper(a.ins, b2.ins, reason="stagger bulk DMA bandwidth")

            if stage < 1:
                lg = dp.tile([NCLS, BPC], F32)
                nc.vector.memset(lg[:], 0.0)
                nc.vector.tensor_copy(lg[0:4 * M, 0:1], sg[:, 0:1])
                nc.vector.tensor_copy(lg[0:2 * M * HEADS - 31, 1:2], at[0][0:2 * M * HEADS - 31, 0:1])
                nc.sync.dma_start(out_h[:], lg[:])
                return _finish(nc)

            # ---- entity embeddings: log-sum-exp over mentions ----
            exps = dp.tile([4 * M, HID], BF16)
            nc.scalar.activation(exps[:], sg[:], AF.Exp)
            ps_e = psb.tile([4, HID], F32, tag="big")
            for n0, nl_ in ((0, 512), (512, 256)):
                nc.tensor.matmul(ps_e[:, n0:n0 + nl_], lhsT=selE,
                                 rhs=exps[:, n0:n0 + nl_], start=True, stop=True)
            ent = dp.tile([4, HID], F32)
            nc.scalar.activation(ent[:], ps_e[:], AF.Ln)
            ps_et = pss.tile([128, 4 * NEMB], F32, tag="sm")
            for c in range(NEMB):
                nc.tensor.transpose(ps_et[:, c * 4:(c + 1) * 4],
                                    ent[:, c * 128:(c + 1) * 128], idf[0:4, 0:4])
            entT = dp.tile([128, 4 * NEMB], BF16)
            nc.vector.tensor_copy(entT[:], ps_et[:])

            # ---- entity attention pooling + context vector ----
            htc = []
            for b in range(BPC):
                ps_eah = pse.tile([HEADS, L], F32, tag="ea")
                nc.tensor.matmul(ps_eah[:], lhsT=selA[:, 0:HEADS], rhs=at[b][:],
                                 start=True, stop=True)
                ps_eat = pse.tile([HEADS, L], F32, tag="ea")
                nc.tensor.matmul(ps_eat[:], lhsT=selA[:, HEADS:2 * HEADS],
                                 rhs=at[b][:], start=True, stop=True)
                eah = dp.tile([HEADS, L], F32, tag=f"eah{b}")
                nc.vector.tensor_copy(eah[:], ps_eah[:])
                prd = dp.tile([HEADS, L], BF16, tag=f"prd{b}")
                nc.vector.tensor_tensor(out=prd[:], in0=eah[:], in1=ps_eat[:],
                                        op=ALU.mult)
                ps_ht = pss.tile([1, L], F32, tag="sm")
                nc.tensor.matmul(ps_ht[:], lhsT=w12, rhs=prd[:],
                                 start=True, stop=True)
                sm = dp.tile([1, 1], F32, tag=f"sm{b}")
                nc.vector.reduce_sum(sm[:], ps_ht[:], axis=mybir.AxisListType.X)
                den = dp.tile([1, 1], F32, tag=f"den{b}")
                nc.vector.tensor_scalar_add(den[:], sm[:], 1e-5)
                rcp = dp.tile([1, 1], F32, tag=f"rcp{b}")
                nc.vector.reciprocal(rcp[:], den[:])
                htn = dp.tile([1, L], F32, tag=f"htn{b}")
                nc.vector.tensor_scalar_mul(htn[:], ps_ht[:], rcp[:, :1])
                ps_htc = pss.tile([128, NL], F32, tag="sm")
                for c in range(NL):
                    nc.tensor.transpose(ps_htc[:, c:c + 1],
                                        htn[:, c * 128:(c + 1) * 128],
                                        idf[0:1, 0:1])
                h = dp.tile([128, NL], BF16, tag=f"htc{b}")
                nc.vector.tensor_copy(h[:], ps_htc[:])
                htc.append(h)

            if stage < 2:
                lg = dp.tile([NCLS, BPC], F32)
                nc.vector.memset(lg[:], 0.0)
                nc.vector.tensor_copy(lg[0:97, 0:1], entT[0:97, 0:1])
                nc.vector.tensor_copy(lg[0:97, 1:2], htc[0][0:97, 0:1])
                nc.sync.dma_start(out_h[:], lg[:])
                return _finish(nc)

            # ---- rs = seq^T @ ht  (column form) ----
            ps_rsc = pss.tile([128, NEMB * BPC], F32, tag="sm")
            for b in range(BPC):
                for d in range(NEMB):
                    for c in range(NL):
                        nc.tensor.matmul(
                            ps_rsc[:, d * BPC + b:d * BPC + b + 1],
                            lhsT=seqt[b][:, c * HID + d * 128:c * HID + (d + 1) * 128],
                            rhs=htc[b][:, c:c + 1],
                            start=(c == 0), stop=(c == NL - 1))
            rsc = dp.tile([128, 4 * NEMB], BF16)
            nc.vector.tensor_copy(
                rsc[:].rearrange("p (r b m) -> p r b m", r=NEMB, b=BPC),
                ps_rsc[:].rearrange("p (r b) -> p r b", r=NEMB)
                .unsqueeze(3).broadcast_to([128, NEMB, BPC, 2]))

            # ---- extractor GEMVs:  [4,768] = cat4^T @ W^T  ----
            def cat_chunk(j):
                if j < NEMB:
                    return entT[:, j * 4:(j + 1) * 4]
                if j < 2 * NEMB:
                    return rsc[:, (j - NEMB) * 4:(j - NEMB + 1) * 4]
                return ner4[:]

            t4 = []
            for wi, (ws, wn, selb, br) in enumerate(
                    ((whs, whn, selbh, bhr), (wts, wtn, selbt, btr))):
                ps_w = psb.tile([4, EMB], F32, tag="big")
                for n0, nl_ in ((0, 512), (512, 256)):
                    for j in range(KCH + 1):
                        lhsT = cat_chunk(j)
                        rhs = (ws[:, j * EMB + n0:j * EMB + n0 + nl_] if j < KCH
                               else wn[:, n0:n0 + nl_])
                        nc.tensor.matmul(ps_w[:, n0:n0 + nl_], lhsT=lhsT, rhs=rhs,
                                         start=(j == 0), stop=False)
                    nc.tensor.matmul(ps_w[:, n0:n0 + nl_], lhsT=selb,
                                     rhs=br[:, n0:n0 + nl_], start=False, stop=True)
                t = dp.tile([4, EMB], F32, tag=f"t4_{wi}")
                nc.scalar.activation(t[:], ps_w[:], AF.Tanh)
                t4.append(t)

            if stage < 3:
                lg = dp.tile([NCLS, BPC], F32)
                nc.vector.memset(lg[:], 0.0)
                nc.vector.tensor_copy(lg[0:4, 0:2], t4[0][:, 0:2])
                nc.vector.tensor_copy(lg[0:4, 1:2], t4[1][:, 0:1])
                nc.vector.tensor_copy(lg[0:89, 0:1], rsc[0:89, 0:1])
                nc.sync.dma_start(out_h[:], lg[:])
                return _finish(nc)

            # ---- transpose hs2/ts2 to columns ----
            ps_a = pss.tile([128, 4 * NEMB], F32, tag="sm")
            ps_b2 = pss.tile([128, 4 * NEMB], F32, tag="sm")
            for c in range(NEMB):
                nc.tensor.transpose(ps_a[:, c * 4:(c + 1) * 4],
                                    t4[0][:, c * 128:(c + 1) * 128], idf[0:4, 0:4])
                nc.tensor.transpose(ps_b2[:, c * 4:(c + 1) * 4],
                                    t4[1][:, c * 128:(c + 1) * 128], idf[0:4, 0:4])
            h2t = dp.tile([128, 4 * NEMB], BF16)
            nc.vector.tensor_copy(
                h2t[:].rearrange("p (c b) -> p c b", c=NEMB)[:, :, 0:4:2],
                ps_a[:].rearrange("p (c b) -> p c b", c=NEMB)[:, :, 0:4:2])
            nc.vector.tensor_copy(
                h2t[:].rearrange("p (c b) -> p c b", c=NEMB)[:, :, 1:4:2],
                ps_b2[:].rearrange("p (c b) -> p c b", c=NEMB)[:, :, 1:4:2])

            if stage < 4:
                lg = dp.tile([NCLS, BPC], F32)
                nc.vector.memset(lg[:], 0.0)
                nc.vector.tensor_copy(lg[0:97, 0:2], h2t[0:97, 0:2])
                nc.sync.dma_start(out_h[:], lg[:])
                return _finish(nc)

            # ---- grouped bilinear + classifier ----
            # ts-replication: out col layout (y, c, b) = y*12 + c*2 + b
            ps_t2x = pss.tile([128, BLK * NEMB * BPC], F32, tag="sm")
            tscols = h2t[:].rearrange("p (c b) -> p c b", c=NEMB)[:, :, 1:4:2]
            for y in range(BLK):
                nc.tensor.matmul(
                    ps_t2x[:, y * 12:(y + 1) * 12]
                    .rearrange("p (c b) -> p c b", c=NEMB),
                    lhsT=rys[:, y * 128:(y + 1) * 128],
                    rhs=tscols, start=True, stop=True)
            if stage < 5:
                lg = dp.tile([NCLS, BPC], F32)
                nc.vector.memset(lg[:], 0.0)
                nc.vector.tensor_copy(lg[0:97, 0:2], ps_t2x[0:97, 0:2])
                nc.sync.dma_start(out_h[:], lg[:])
                return _finish(nc)

            blt = dp.tile([128, NEMB * 16], BF16)
            for c in range(NEMB):
                nc.vector.tensor_tensor(
                    out=blt[:, c * 16:(c + 1) * 16]
                    .rearrange("p (y b) -> p y b", y=BLK),
                    in0=h2t[:, c * 4:c * 4 + 4:2].unsqueeze(1)
                        .broadcast_to([128, BLK, 2]),
                    in1=ps_t2x[:].rearrange("p (y c b) -> p y c b", y=BLK, c=NEMB)
                    [:, :, c, :],
                    op=ALU.mult)
            if stage < 6:
                lg = dp.tile([NCLS, BPC], F32)
                nc.vector.memset(lg[:], 0.0)
                nc.vector.tensor_copy(lg[0:97, 0:2], blt[0:97, 0:2])
                nc.sync.dma_start(out_h[:], lg[:])
                return _finish(nc)

            ps_l = pss.tile([NCLS, BPC], F32, tag="sm")
            for c in range(NEMB):
                for y in range(BLK):
                    k = c * BLK + y
                    nc.tensor.matmul(ps_l[:], lhsT=wbs[:, k * 128:k * 128 + NCLS],
                                     rhs=blt[:, c * 16 + y * 2:c * 16 + y * 2 + 2],
                                     start=(k == 0), stop=(k == NBL - 1))
            lg = dp.tile([NCLS, BPC], F32)
            if stage < 7:
                nc.vector.memset(lg[:], 0.0)
                nc.vector.tensor_copy(lg[0:1, 0:1], ps_l[0:1, 0:1])
            else:
                nc.vector.tensor_scalar_add(lg[:], ps_l[:], bbc[:, :1])
            nc.sync.dma_start(out_h[:], lg[:])

    return _finish(nc)


def _finish(nc):
    return nc


def _get_program():
    if "nc" not in _cache:
        nc = _build_program()
        nc.finalize()
        _cache["nc"] = nc
        _cache["consts"] = _build_constants()
    return _cache["nc"], _cache["consts"]


def kernel(sequence_output, attention, entity_pos, hs_ner_tags, ts_ner_tags,
           Wh, bh, Wt, bt, Wb, bb):
    nc, c = _get_program()

    seq = np.asarray(sequence_output, dtype=np.float32).astype(ml_dtypes.bfloat16)
    attn = np.asarray(attention, dtype=np.float32).astype(ml_dtypes.bfloat16)
    pos = np.asarray(entity_pos).astype(np.int32)
    nh = np.asarray(hs_ner_tags, dtype=np.float32)
    nt = np.asarray(ts_ner_tags, dtype=np.float32)
    whT = np.ascontiguousarray(np.asarray(Wh, dtype=np.float32).T).astype(ml_dtypes.bfloat16)
    wtT = np.ascontiguousarray(np.asarray(Wt, dtype=np.float32).T).astype(ml_dtypes.bfloat16)
    wbT = np.ascontiguousarray(np.asarray(Wb, dtype=np.float32).T)[c["perm"]]
    wbT = np.pad(wbT, ((0, 0), (0, 128 - NCLS))).astype(ml_dtypes.bfloat16)

    def sbuf_image(w, extra):
        main = w[0:KCH * 128].reshape(KCH, 128, EMB).transpose(1, 0, 2).reshape(128, KCH * EMB)
        img = np.zeros((128, KCH * EMB + EMB), ml_dtypes.bfloat16)
        img[:, 0:KCH * EMB] = main
        img[0:NER, KCH * EMB:] = extra
        return img

    whs = sbuf_image(whT, whT[KCH * 128:CAT])
    wts = sbuf_image(wtT, wtT[KCH * 128:CAT])
    wbs = wbT.reshape(NBL, 128, 128).transpose(1, 0, 2).reshape(128, NBL * 128)
    wbs = np.ascontiguousarray(wbs)

    c["cbb"][0:1, BHR0:BHR0 + EMB] = np.asarray(bh, np.float32).reshape(1, EMB)
    c["cbb"][0:1, BTR0:BTR0 + EMB] = np.asarray(bt, np.float32).reshape(1, EMB)
    cid2 = c["cid2"].copy()
    cid2[0:97, 128] = np.asarray(bb, np.float32)

    in_maps = []
    for core in range(NCORES):
        b0 = core * BPC
        pc = np.ascontiguousarray(pos[b0:b0 + BPC])          # [2,2,M]
        ner = np.stack([nh[b0], nt[b0], nh[b0 + 1], nt[b0 + 1]], axis=1)
        im = {
            "seq": np.ascontiguousarray(seq[b0:b0 + BPC]).reshape(BPC * L, HID),
            "attn": np.ascontiguousarray(attn[b0:b0 + BPC]).reshape(BPC * HEADS * L, L),
            "pos": pc.reshape(4 * M, 1),
            "ner": np.ascontiguousarray(ner.astype(np.float32)),
            "whs": whs, "wts": wts, "wbs": wbs,
            "cidx": c["cidx"], "cid2": cid2, "cba": c["cba"], "cbb": c["cbb"],
        }
        for b in range(BPC):
            im[f"posb{b}"] = np.ascontiguousarray(pc[b].reshape(2 * M, 1))
        in_maps.append(im)

    res = run_bass_kernel_spmd(nc, in_maps, core_ids=list(range(NCORES)))
    _cache["last_res"] = res
    out = np.empty((B, NCLS), np.float32)
    for core in range(NCORES):
        out[core * BPC:(core + 1) * BPC] = res.results[core]["logitsT"].T
    return out
